# revision 2
# baseline (speedup 1.0000x reference)
"""Causal attention (B=4, L=4096, D=2048, HD=128) on 8 TRN2 NeuronCores.

Sharding: 8 cores = 4 batches x 2 fold-halves. Core c handles batch b=c//2
and query blocks {i, 3-i} (1024 rows each) where i=c%2 — the "fold" split
balances causal attention work exactly across the two cores of a batch.
Each core recomputes K/V for all 4096 keys of its batch (no collectives).

The on-device program is identical on all cores (SPMD); per-core behavior
comes only from the data: a block-permuted transposed input xT and two
slot-bias vectors that enable/disable the two fold-dependent key blocks
(bias 0 keeps scores, bias -50 drives exp() to ~1e-22, i.e. masks).

Layouts (partition dim first):
  xT      [D=2048, 4096]   x[b].T with key blocks permuted to local order
  Qt, Kt  [HD=128, Lq/Lk]  projections, head dim on partitions
  V_aug   [k, HD+1]        natural V with a ones column -> fused row-sums
  scores  [k=128, q=512]   St = Kt_tile^T-free matmul; exp on ACT engine
  AV out  [q=128, 129]     col 128 = softmax denominator
  outT    [D, q]           final projection, transposed; host transposes back
"""

import numpy as np

B, L, D, HD = 4, 4096, 2048, 128
BLK = 1024            # fold block (4 per batch)
LQ = 2 * BLK          # queries per core
LK = L                # keys per core
ND = D // 128         # 16 d-tiles
NRB = LK // 512       # 8 column blocks for projections
NEG = -50.0           # slot-disable bias (exp(x-50) ~ 0)
MASKVAL = -30000.0    # intra-tile causal mask additive value

_cached = {}


def _build_program():
    import concourse.bass as bass
    import concourse.tile as tile
    from concourse import bacc, mybir
    from concourse.masks import make_identity

    f32 = mybir.dt.float32
    nc = bacc.Bacc("TRN2", target_bir_lowering=False, debug=False)

    xT_d = nc.dram_tensor("xT", (D, LK), f32, kind="ExternalInput")
    wq_d = nc.dram_tensor("wq", (D, HD), f32, kind="ExternalInput")
    wk_d = nc.dram_tensor("wk", (D, HD), f32, kind="ExternalInput")
    wv_d = nc.dram_tensor("wv", (D, HD), f32, kind="ExternalInput")
    wo_d = nc.dram_tensor("wo", (HD, D), f32, kind="ExternalInput")
    bias_d = nc.dram_tensor("biases", (128, 8), f32, kind="ExternalInput")
    bo_d = nc.dram_tensor("bo_t", (128, ND), f32, kind="ExternalInput")
    out_d = nc.dram_tensor("outT", (D, LQ), f32, kind="ExternalOutput")

    # phase -> list of (local_kblk, kind); kind in {"diag", "full", "bA", "bB"}
    SLOTS = {
        0: [(0, "diag"), (2, "bA")],
        1: [(0, "full"), (1, "diag"), (2, "full"), (3, "bB")],
    }

    with tile.TileContext(nc) as tc:
        with (
            tc.tile_pool(name="const", bufs=1) as cpool,
            tc.tile_pool(name="xt", bufs=2) as xtpool,
            tc.tile_pool(name="vt", bufs=2) as vtpool,
            tc.tile_pool(name="expst", bufs=4) as epool,
            tc.tile_pool(name="otile", bufs=4) as opool,
            tc.tile_pool(name="outsb", bufs=2) as outpool,
        ):
            # ---- persistent SBUF tensors ----
            wq_s = cpool.tile([128, ND, 128], f32, tag="wq")
            wk_s = cpool.tile([128, ND, 128], f32, tag="wk")
            wv_s = cpool.tile([128, ND, 128], f32, tag="wv")
            wo_s = cpool.tile([128, D], f32, tag="wo")
            bias_s = cpool.tile([128, 8], f32, tag="biases")
            bo_s = cpool.tile([128, ND], f32, tag="bo")
            kt_s = cpool.tile([128, LK], f32, tag="kt")
            qt_s = cpool.tile([128, LQ], f32, tag="qt")
            vaug_s = cpool.tile([128, (LK // 128) * (HD + 1)], f32, tag="vaug")
            masks_s = cpool.tile([128, 4 * 512], f32, tag="masks")
            ot_s = cpool.tile([128, LQ], f32, tag="ot")
            ident_s = cpool.tile([128, 128], f32, tag="ident")

            for w_d, w_s in ((wq_d, wq_s), (wk_d, wk_s), (wv_d, wv_s)):
                nc.sync.dma_start(
                    w_s[:], w_d.ap().rearrange("(n p) m -> p n m", p=128)
                )
            nc.sync.dma_start(wo_s[:], wo_d.ap())
            nc.sync.dma_start(bias_s[:], bias_d.ap())
            nc.sync.dma_start(bo_s[:], bo_d.ap())

            make_identity(nc, ident_s[:])
            # ones column for V_aug (col HD of each 129-wide group)
            nc.gpsimd.memset(vaug_s[:], 1.0)
            # 4 causal mask tiles for relative offsets delta = 0,128,256,384:
            # keep 0 where q_free >= k_part + delta, else MASKVAL
            nc.gpsimd.memset(masks_s[:], 0.0)
            for m in range(4):
                nc.gpsimd.affine_select(
                    out=masks_s[:, m * 512:(m + 1) * 512],
                    in_=masks_s[:, m * 512:(m + 1) * 512],
                    compare_op=mybir.AluOpType.is_ge,
                    fill=MASKVAL,
                    base=-(m * 128),
                    channel_multiplier=-1,
                    pattern=[[1, 512]],
                )

            bq_ap = bias_s[:, 0:1]
            bk_ap = bias_s[:, 1:2]
            bv_ap = bias_s[:, 2:3]
            slot_bias = {"bA": bias_s[:, 3:4], "bB": bias_s[:, 4:5]}

            # ---- phase 1: projections Qt, Kt, V_aug ----
            with (
                tc.tile_pool(name="ppsum", bufs=2, space="PSUM") as ppsum,
                tc.tile_pool(name="vtpsum", bufs=2, space="PSUM") as vtpsum,
            ):
                xT_r = xT_d.ap().rearrange("(n p) m -> p n m", p=128)
                for rb in range(NRB):
                    xt = xtpool.tile([128, ND, 512], f32, tag="xt")
                    nc.sync.dma_start(
                        xt[:], xT_r[:, :, rb * 512:(rb + 1) * 512]
                    )
                    cs = slice(rb * 512, (rb + 1) * 512)

                    pk = ppsum.tile([128, 512], f32, tag="pk")
                    for dt in range(ND):
                        nc.tensor.matmul(
                            pk[:], wk_s[:, dt, :], xt[:, dt, :],
                            start=(dt == 0), stop=(dt == ND - 1),
                        )
                    nc.vector.tensor_scalar_add(kt_s[:, cs], pk[:], bk_ap)

                    pv = ppsum.tile([128, 512], f32, tag="pv")
                    for dt in range(ND):
                        nc.tensor.matmul(
                            pv[:], wv_s[:, dt, :], xt[:, dt, :],
                            start=(dt == 0), stop=(dt == ND - 1),
                        )
                    vt_tmp = vtpool.tile([128, 512], f32, tag="vt_tmp")
                    nc.vector.tensor_scalar_add(vt_tmp[:], pv[:], bv_ap)
                    for s in range(4):
                        ktile = rb * 4 + s
                        vp = vtpsum.tile([128, 128], f32, tag="vp")
                        nc.tensor.transpose(
                            vp[:], vt_tmp[:, s * 128:(s + 1) * 128], ident_s[:]
                        )
                        nc.vector.tensor_copy(
                            vaug_s[:, ktile * 129: ktile * 129 + 128], vp[:]
                        )

                    if rb < LQ // 512:
                        pq = ppsum.tile([128, 512], f32, tag="pq")
                        for dt in range(ND):
                            nc.tensor.matmul(
                                pq[:], wq_s[:, dt, :], xt[:, dt, :],
                                start=(dt == 0), stop=(dt == ND - 1),
                            )
                        nc.vector.tensor_scalar_add(qt_s[:, cs], pq[:], bq_ap)

            # ---- phase 2: attention ----
            with (
                tc.tile_pool(name="stpsum", bufs=2, space="PSUM") as stpsum,
                tc.tile_pool(name="avpsum", bufs=4, space="PSUM") as avpsum,
                tc.tile_pool(name="trpsum", bufs=2, space="PSUM") as trpsum,
            ):
                for phase in (0, 1):
                    for u in range(2):
                        q0 = phase * BLK + u * 512
                        # build the static k-tile list for this q sub-block
                        klist = []  # (ktile_global, mask_idx or None, bias_key)
                        for kblk, kind in SLOTS[phase]:
                            for t in range(8):
                                if kind == "diag":
                                    drel = t * 128 - u * 512
                                    if drel >= 512:
                                        continue
                                    midx = drel // 128 if drel >= 0 else None
                                    klist.append((kblk * 8 + t, midx, None))
                                else:
                                    bkey = kind if kind in slot_bias else None
                                    klist.append((kblk * 8 + t, None, bkey))

                        avs = [
                            avpsum.tile([128, HD + 1], f32, tag="av", name="av")
                            for _ in range(4)
                        ]
                        for ki, (kt, midx, bkey) in enumerate(klist):
                            st = stpsum.tile([128, 512], f32, tag="st")
                            nc.tensor.matmul(
                                st[:],
                                kt_s[:, kt * 128:(kt + 1) * 128],
                                qt_s[:, q0:q0 + 512],
                                start=True, stop=True,
                            )
                            if midx is not None:
                                nc.vector.tensor_add(
                                    st[:], st[:],
                                    masks_s[:, midx * 512:(midx + 1) * 512],
                                )
                            est = epool.tile([128, 512], f32, tag="est")
                            nc.scalar.activation(
                                est[:], st[:],
                                mybir.ActivationFunctionType.Exp,
                                bias=slot_bias[bkey] if bkey else 0.0,
                            )
                            first, last = ki == 0, ki == len(klist) - 1
                            for v in range(4):
                                nc.tensor.matmul(
                                    avs[v][:],
                                    est[:, v * 128:(v + 1) * 128],
                                    vaug_s[:, kt * 129: kt * 129 + 129],
                                    start=first, stop=last,
                                )
                        for v in range(4):
                            otile = opool.tile([128, HD], f32, tag="otile")
                            recip = opool.tile([128, 1], f32, tag="recip")
                            nc.vector.reciprocal(
                                recip[:], avs[v][:, HD:HD + 1]
                            )
                            nc.vector.tensor_scalar_mul(
                                otile[:], avs[v][:, 0:HD], recip[:]
                            )
                            tp = trpsum.tile([128, 128], f32, tag="tr")
                            nc.tensor.transpose(tp[:], otile[:], ident_s[:])
                            qc = q0 + v * 128
                            nc.vector.tensor_copy(ot_s[:, qc:qc + 128], tp[:])

            # ---- phase 3: output projection ----
            with tc.tile_pool(name="opsum", bufs=2, space="PSUM") as opsum:
                for dt in range(ND):
                    orow = outpool.tile([128, LQ], f32, tag="orow")
                    for qb in range(LQ // 512):
                        po = opsum.tile([128, 512], f32, tag="po")
                        nc.tensor.matmul(
                            po[:],
                            wo_s[:, dt * 128:(dt + 1) * 128],
                            ot_s[:, qb * 512:(qb + 1) * 512],
                            start=True, stop=True,
                        )
                        nc.vector.tensor_scalar_add(
                            orow[:, qb * 512:(qb + 1) * 512], po[:],
                            bo_s[:, dt:dt + 1],
                        )
                    nc.sync.dma_start(
                        out_d.ap()[dt * 128:(dt + 1) * 128, :], orow[:]
                    )

    nc.compile()
    return nc


def _get_program():
    if "nc" not in _cached:
        _cached["nc"] = _build_program()
    return _cached["nc"]


def _perm_blocks(i):
    # local order [qA, qB, o1, o2]
    return [0, 3, 1, 2] if i == 0 else [1, 2, 0, 3]


def make_in_maps(x, Wq, bq, Wk, bk, Wv, bv, Wo, bo):
    scale = 1.0 / np.sqrt(np.float32(HD))
    wq_s = (Wq * scale).astype(np.float32)
    bq_s = (bq * scale).astype(np.float32)
    bo_t = np.ascontiguousarray(
        bo.astype(np.float32).reshape(ND, 128).T
    )  # [128, ND]
    in_maps = []
    for c in range(8):
        i, b = c % 2, c // 2
        perm = _perm_blocks(i)
        xbT = x[b].T  # (D, L) view
        xT = np.concatenate(
            [xbT[:, p * BLK:(p + 1) * BLK] for p in perm], axis=1
        ).astype(np.float32)
        biases = np.zeros((128, 8), np.float32)
        biases[:, 0] = bq_s
        biases[:, 1] = bk.astype(np.float32)
        biases[:, 2] = bv.astype(np.float32)
        biases[:, 3] = NEG if i == 0 else 0.0   # phase A, slot kblk=2
        biases[:, 4] = 0.0 if i == 0 else NEG   # phase B, slot kblk=3
        in_maps.append({
            "xT": np.ascontiguousarray(xT),
            "wq": wq_s,
            "wk": Wk.astype(np.float32),
            "wv": Wv.astype(np.float32),
            "wo": Wo.astype(np.float32),
            "biases": biases,
            "bo_t": bo_t,
        })
    return in_maps


def assemble_output(results):
    out = np.empty((B, L, D), np.float32)
    for c in range(8):
        i, b = c % 2, c // 2
        perm = _perm_blocks(i)
        outT = results[c]["outT"]  # (D, LQ)
        qA, qB = perm[0], perm[1]
        out[b, qA * BLK:(qA + 1) * BLK, :] = outT[:, 0:BLK].T
        out[b, qB * BLK:(qB + 1) * BLK, :] = outT[:, BLK:2 * BLK].T
    return out


def kernel(x, Wq, bq, Wk, bk, Wv, bv, Wo, bo):
    from concourse.bass_utils import run_bass_kernel_spmd

    nc = _get_program()
    in_maps = make_in_maps(
        np.asarray(x), np.asarray(Wq), np.asarray(bq), np.asarray(Wk),
        np.asarray(bk), np.asarray(Wv), np.asarray(bv), np.asarray(Wo),
        np.asarray(bo),
    )
    res = run_bass_kernel_spmd(nc, in_maps, core_ids=list(range(8)))
    return assemble_output(res.results)
